# revision 2
# baseline (speedup 1.0000x reference)
"""Multi-head self-attention (B=2, T=4096, D=768, H=12) on 8 TRN2 NeuronCores.

Sharding: (batch, head)-parallel. Core c (0..7) handles batch b=c//4 and the
3 heads h0=(c%4)*3 .. h0+2.  Each core computes Q/K/V projections for its
heads, full softmax(QK^T/sqrt(d))V attention, and a partial output projection
through its 192 rows of Wo.  The host sums the 4 partials per batch and adds
the output bias bo.

Per-core pipeline (v2):
  Phase 1: Q/K projections into [d, t] layout (head pair packed on 128
  partitions; h2's 64-dim blocks of Q and K col-tiled into one PSUM bank);
  V in natural [t, d] layout augmented with a trailing ones column (row 64 of
  the PV output then accumulates the softmax denominator).
  Phase 2, per 512-wide q tile: for each 128-wide k tile, a row-tiled pair of
  score matmuls (S^T[k,q] for h0/h1 concurrently); exp alternates between the
  ACT engine (exact, PSUM->SBUF bf16) and the DVE (Schraudolph bitcast exp:
  i16 = round(23.083*s + 16250.5) whose bits are bf16 e^(s/8), ~3% pointwise,
  ~0.2% on the softmax output); PV accumulates O^T[65,512] per head with the
  denominator in row 64.  h2 is processed with its Q/K duplicated onto both
  row halves, two k tiles per step.  Normalization divides by row 64 via a
  K=1 broadcast matmul + fast reciprocal.  The Wo projection consumes the
  merged normalized tile on01 [128, t] (h0 rows 0:64, h1 rows 64:128) and
  on2 [64, t] in two accumulating matmuls per output block; y is written bf16.
"""

import os
import numpy as np
import ml_dtypes

B, T, D = 2, 4096, 768
H, DH = 12, 64
NCORES = 8
HPC = 3            # heads per core
KC = D // 128      # 6 contraction chunks for projections
NT = T // 512      # 8 q tiles of 512
TT = T // 128      # 32 k tiles of 128

# Schraudolph constants: i16 = round(s * SCH_A + SCH_B); bits viewed as bf16
# give e^(s/8) with ~3.3% max pointwise error.
SCH_A = 128 / float(np.log(2)) * 0.125   # 23.08312065
SCH_B = 16256.0 - 5.5

BF16 = ml_dtypes.bfloat16

_CACHE = {}


def _trace(nc, tc, mybir, tens, iters=1):
    import concourse.bass as bass
    from contextlib import ExitStack

    f32 = mybir.dt.float32
    bf16 = mybir.dt.bfloat16
    i16 = mybir.dt.int16
    f32r = mybir.dt.float32r
    Exp = mybir.ActivationFunctionType.Exp
    PSUM = bass.MemorySpace.PSUM
    Mult = mybir.AluOpType.mult
    Add = mybir.AluOpType.add

    with ExitStack() as ctx:
        persist = ctx.enter_context(tc.tile_pool(name="persist", bufs=1))

        # ---- persistent SBUF ----
        x_ch = [
            persist.tile([128, T], bf16, name=f"xc{kc}") for kc in range(KC)
        ]
        w_q = persist.tile([128, KC, HPC * DH], bf16)
        w_k = persist.tile([128, KC, HPC * DH], bf16)
        w_v = persist.tile([128, KC, HPC * DH], bf16)
        bq01 = persist.tile([128, 1], f32)
        bq2 = persist.tile([64, 1], f32)
        bk01 = persist.tile([128, 1], f32)
        bk2 = persist.tile([64, 1], f32)
        bv_sb = persist.tile([1, HPC * DH], bf16)
        ones1 = persist.tile([1, 128], bf16)     # K=1 lhsT for V bias MM
        ones65 = persist.tile([DH + 1, DH + 1], f32r)  # row 64: K=1 denom bcast lhsT
        q01 = persist.tile([128, T], bf16)       # h0 rows 0:64, h1 rows 64:128
        k01 = persist.tile([128, T], bf16)
        q2 = persist.tile([128, T], bf16)        # h2, duplicated to rows 64:128
        k2 = persist.tile([128, T], bf16)
        v_sb = persist.tile([128, TT, HPC, 68], bf16)  # [V|1] per head
        # normalized O^T: h0 rows 0:64, h1 rows 64:128; h2 separate
        on01 = persist.tile([128, T], bf16)
        on2 = persist.tile([DH, T], bf16)
        wo01_sb = persist.tile([128, D], bf16)
        wo2_sb = persist.tile([DH, D], bf16)

        nc.vector.memset(v_sb[:, :, :, 64:65], 1.0)

        # ---- input DMAs ----
        xT, wqT, wkT, wvT, bq, bk, bv, wo01, wo2, onesb, ones65d, y = tens
        nc.sync.dma_start(ones1[:], onesb[0:1, 0:128])
        nc.sync.dma_start(ones65[DH : DH + 1, :], ones65d[:])
        for kc in range(KC):
            r = slice(kc * 128, (kc + 1) * 128)
            nc.sync.dma_start(x_ch[kc][:], xT[r, :])
            nc.sync.dma_start(w_q[:, kc, :], wqT[r, :])
            nc.sync.dma_start(w_k[:, kc, :], wkT[r, :])
            nc.sync.dma_start(w_v[:, kc, :], wvT[r, :])
        nc.sync.dma_start(bq01[:], bq[0:128, :])
        nc.sync.dma_start(bq2[:], bq[128:192, :])
        nc.sync.dma_start(bk01[:], bk[0:128, :])
        nc.sync.dma_start(bk2[:], bk[128:192, :])
        nc.sync.dma_start(bv_sb[:], bv[:])
        nc.sync.dma_start(wo01_sb[:], wo01[:])
        nc.sync.dma_start(wo2_sb[:], wo2[:])

        loop_cm = tc.For_i(0, iters, 1) if iters > 1 else None
        from contextlib import nullcontext
        with (loop_cm if loop_cm is not None else nullcontext()):
            # ---- Phase 1a: Q/K projections into [d, t] layout ----
            with tc.tile_pool(name="pj", bufs=2, space=PSUM) as pj:
                for nt in range(NT):
                    s = slice(nt * 512, (nt + 1) * 512)
                    pqa = pj.tile([128, 512], f32, tag="pqa")
                    pka = pj.tile([128, 512], f32, tag="pka")
                    pb = pj.tile([128, 512], f32, tag="pb")  # q2 | k2 col-tiled
                    for kc in range(KC):
                        st, sp = kc == 0, kc == KC - 1
                        rhs = x_ch[kc][:, s]
                        nc.tensor.matmul(pqa[:], w_q[:, kc, 0:128], rhs, start=st, stop=sp)
                        nc.tensor.matmul(pka[:], w_k[:, kc, 0:128], rhs, start=st, stop=sp)
                        nc.tensor.matmul(pb[0:64, :], w_q[:, kc, 128:192], rhs,
                                         start=st, stop=sp, tile_position=(0, 0),
                                         skip_group_check=True)
                        nc.tensor.matmul(pb[64:128, :], w_k[:, kc, 128:192], rhs,
                                         start=st, stop=sp, tile_position=(0, 64),
                                         skip_group_check=True)
                    nc.vector.tensor_scalar_add(q01[:, s], pqa[:], bq01[:])
                    nc.vector.tensor_scalar_add(k01[:, s], pka[:], bk01[:])
                    nc.vector.tensor_scalar_add(q2[0:64, s], pb[0:64, :], bq2[:])
                    nc.vector.tensor_scalar_add(k2[0:64, s], pb[64:128, :], bk2[:])

            # ---- Phase 1b: V projection into natural [t, d] layout ----
            with tc.tile_pool(name="pv", bufs=4, space=PSUM) as pvp:
                for tt in range(TT):
                    ts_ = slice(tt * 128, (tt + 1) * 128)
                    pvt = pvp.tile([128, HPC * DH], f32, tag="pvt")
                    nc.tensor.matmul(pvt[:], ones1[:], bv_sb[:], start=True, stop=False)
                    for kc in range(KC):
                        nc.tensor.matmul(
                            pvt[:], x_ch[kc][:, ts_], w_v[:, kc, :],
                            start=False, stop=kc == KC - 1,
                        )
                    nc.vector.tensor_copy(
                        v_sb[:, tt, :, 0:64],
                        pvt[:].rearrange("p (h d) -> p h d", h=HPC),
                    )

            # duplicate h2's Q/K to partitions 64..127 for self-paired row tiling
            nc.sync.dma_start(q2[64:128, :], q2[0:64, :])
            nc.sync.dma_start(k2[64:128, :], k2[0:64, :])

            # ---- Phase 2: attention + output projection, per q tile ----
            with (
                tc.tile_pool(name="spool", bufs=2, space=PSUM) as spool,
                tc.tile_pool(name="opool", bufs=1, space=PSUM) as opool,
                tc.tile_pool(name="mpool", bufs=2, space=PSUM) as mpool,
                tc.tile_pool(name="ppool", bufs=3) as ppool,
                tc.tile_pool(name="npool", bufs=2) as npool,
                tc.tile_pool(name="ypool", bufs=2) as ypool,
            ):
                for qt in range(NT):
                    qs = slice(qt * 512, (qt + 1) * 512)

                    def exp_jobs(st_pair, act_idx):
                        """Two [128,512] exp jobs; act_idx goes to ACT, other to DVE."""
                        pt = [
                            ppool.tile([128, 512], bf16, tag=f"p{i}", name=f"p{i}")
                            for i in (0, 1)
                        ]
                        d_idx = 1 - act_idx
                        nc.scalar.activation(pt[act_idx][:], st_pair[act_idx][:],
                                             Exp, scale=0.125)
                        nc.vector.tensor_scalar(pt[d_idx][:].bitcast(i16),
                                                st_pair[d_idx][:],
                                                SCH_A, SCH_B, Mult, Add)
                        return pt

                    # --- h0/h1 concurrently (row strips 0:64 / 64:128) ---
                    ol = [opool.tile([DH + 1, 512], f32, tag=f"o{i}", name=f"ol{i}")
                          for i in (0, 1)]
                    for kt in range(TT):
                        ks = slice(kt * 128, (kt + 1) * 128)
                        st = [spool.tile([128, 512], f32, tag=f"s{i}", name=f"s{i}")
                              for i in (0, 1)]
                        nc.tensor.matmul(st[0][:], k01[0:64, ks], q01[0:64, qs],
                                         start=True, stop=True)
                        nc.tensor.matmul(st[1][:], k01[64:128, ks], q01[64:128, qs],
                                         start=True, stop=True)
                        pt = exp_jobs(st, kt % 2)
                        first, last = kt == 0, kt == TT - 1
                        nc.tensor.matmul(ol[0][:], v_sb[:, kt, 0, 0:65], pt[0][:],
                                         start=first, stop=last, skip_group_check=True)
                        nc.tensor.matmul(ol[1][:], v_sb[:, kt, 1, 0:65], pt[1][:],
                                         start=first, stop=last, skip_group_check=True)

                    # --- normalize h0/h1 into merged on01 ---
                    def norm(o_acc, out_ap):
                        lrow = npool.tile([DH + 1, 512], f32r, tag="lr")
                        nc.vector.tensor_copy(lrow[DH : DH + 1, :], o_acc[DH : DH + 1, :])
                        bc = mpool.tile([DH + 1, 512], f32, tag="y")
                        nc.tensor.matmul(bc[:], ones65[DH : DH + 1, :],
                                         lrow[DH : DH + 1, :], start=True, stop=True)
                        rc = npool.tile([DH + 1, 512], f32, tag="rc")
                        nc.vector.reciprocal_approx_fast(rc[:], bc[:])
                        nc.vector.tensor_mul(out_ap, o_acc[0:DH, :], rc[0:DH, :])

                    norm(ol[0], on01[0:DH, qs])
                    norm(ol[1], on01[DH:128, qs])

                    # --- h2: two k tiles per step on row halves ---
                    o2 = [opool.tile([DH + 1, 512], f32, tag=f"o{i}", name=f"o2_{i}")
                          for i in (0, 1)]
                    for p in range(TT // 2):
                        ka = slice(2 * p * 128, (2 * p + 1) * 128)
                        kb = slice((2 * p + 1) * 128, (2 * p + 2) * 128)
                        st = [spool.tile([128, 512], f32, tag=f"s{i}", name=f"s2{i}")
                              for i in (0, 1)]
                        nc.tensor.matmul(st[0][:], k2[0:64, ka], q2[0:64, qs],
                                         start=True, stop=True)
                        nc.tensor.matmul(st[1][:], k2[64:128, kb], q2[64:128, qs],
                                         start=True, stop=True)
                        pt = exp_jobs(st, p % 2)
                        first, last = p == 0, p == TT // 2 - 1
                        nc.tensor.matmul(o2[0][:], v_sb[:, 2 * p, 2, 0:65], pt[0][:],
                                         start=first, stop=last, skip_group_check=True)
                        nc.tensor.matmul(o2[1][:], v_sb[:, 2 * p + 1, 2, 0:65], pt[1][:],
                                         start=first, stop=last, skip_group_check=True)

                    o2b = npool.tile([DH + 1, 512], f32, tag="o2b")
                    nc.vector.tensor_copy(o2b[:], o2[1][:])
                    o2s = npool.tile([DH + 1, 512], f32, tag="o2s")
                    nc.vector.tensor_add(o2s[:], o2[0][:], o2b[:])
                    norm(o2s, on2[0:DH, qs])

                    # --- output projection for this q tile ---
                    for tt4 in range(4):
                        t0 = qt * 512 + tt4 * 128
                        ts_ = slice(t0, t0 + 128)
                        ysb = ypool.tile([128, D], bf16, tag="ysb")
                        for m0, mw in ((0, 512), (512, 256)):
                            ms = slice(m0, m0 + mw)
                            yps = mpool.tile([128, 512], f32, tag="y")
                            nc.tensor.matmul(yps[:, 0:mw], on01[:, ts_], wo01_sb[:, ms],
                                             start=True, stop=False)
                            nc.tensor.matmul(yps[:, 0:mw], on2[:, ts_], wo2_sb[:, ms],
                                             start=False, stop=True)
                            nc.vector.tensor_copy(ysb[:, ms], yps[:, 0:mw])
                        nc.sync.dma_start(y[ts_, :], ysb[:])


def _build(iters=1):
    import concourse.bacc as bacc
    import concourse.tile as tile
    from concourse import mybir

    f32 = mybir.dt.float32
    bf16 = mybir.dt.bfloat16
    nc = bacc.Bacc("TRN2", target_bir_lowering=False, debug=False, name="mhsa")

    tens = (
        nc.dram_tensor("xT", [D, T], bf16, kind="ExternalInput"),
        nc.dram_tensor("wqT", [D, HPC * DH], bf16, kind="ExternalInput"),
        nc.dram_tensor("wkT", [D, HPC * DH], bf16, kind="ExternalInput"),
        nc.dram_tensor("wvT", [D, HPC * DH], bf16, kind="ExternalInput"),
        nc.dram_tensor("bq", [HPC * DH, 1], f32, kind="ExternalInput"),
        nc.dram_tensor("bk", [HPC * DH, 1], f32, kind="ExternalInput"),
        nc.dram_tensor("bv", [1, HPC * DH], bf16, kind="ExternalInput"),
        nc.dram_tensor("wo01", [128, D], bf16, kind="ExternalInput"),
        nc.dram_tensor("wo2", [DH, D], bf16, kind="ExternalInput"),
        nc.dram_tensor("onesb", [1, T], bf16, kind="ExternalInput"),
        nc.dram_tensor("ones65", [1, DH + 1], mybir.dt.float32r, kind="ExternalInput"),
        nc.dram_tensor("y", [T, D], bf16, kind="ExternalOutput"),
    )
    with tile.TileContext(nc) as tc:
        _trace(nc, tc, mybir, tens, iters)
    nc.finalize()
    return nc


def _prep_inputs(x, Wq, bq, Wk, bk, Wv, bv, Wo, bo):
    in_maps = []
    xTb = [np.ascontiguousarray(x[b].T).astype(BF16) for b in range(B)]
    for c in range(NCORES):
        b = c // 4
        h0 = (c % 4) * HPC
        cols = slice(h0 * DH, (h0 + HPC) * DH)
        woT = np.ascontiguousarray(Wo[:, cols].T)  # [192, 768]
        wo01 = np.ascontiguousarray(woT[0:128]).astype(BF16)
        wo2 = np.ascontiguousarray(woT[128:192]).astype(BF16)
        in_maps.append(
            {
                "xT": xTb[b],
                "wqT": np.ascontiguousarray(Wq[cols, :].T).astype(BF16),
                "wkT": np.ascontiguousarray(Wk[cols, :].T).astype(BF16),
                "wvT": np.ascontiguousarray(Wv[cols, :].T).astype(BF16),
                "bq": np.ascontiguousarray(bq[cols]).reshape(-1, 1).astype(np.float32),
                "bk": np.ascontiguousarray(bk[cols]).reshape(-1, 1).astype(np.float32),
                "bv": np.ascontiguousarray(bv[cols]).reshape(1, -1).astype(BF16),
                "wo01": wo01,
                "wo2": wo2,
                "onesb": np.ones((1, T), dtype=BF16),
                "ones65": np.ones((1, DH + 1), dtype=np.float32),
            }
        )
    return in_maps


def kernel(x, Wq, bq, Wk, bk, Wv, bv, Wo, bo):
    x = np.asarray(x, dtype=np.float32)
    Wq, bq = np.asarray(Wq, np.float32), np.asarray(bq, np.float32)
    Wk, bk = np.asarray(Wk, np.float32), np.asarray(bk, np.float32)
    Wv, bv = np.asarray(Wv, np.float32), np.asarray(bv, np.float32)
    Wo, bo = np.asarray(Wo, np.float32), np.asarray(bo, np.float32)

    from concourse.bass_utils import run_bass_kernel_spmd

    iters = int(os.environ.get("MHSA_ITERS", "1"))
    key = ("nc", iters)
    if key not in _CACHE:
        _CACHE[key] = _build(iters)
    nc = _CACHE[key]

    in_maps = _prep_inputs(x, Wq, bq, Wk, bk, Wv, bv, Wo, bo)
    trace = bool(os.environ.get("MHSA_TRACE"))
    res = run_bass_kernel_spmd(
        nc, in_maps, core_ids=list(range(NCORES)), trace=trace
    )
    if res.exec_time_ns is not None:
        print(f"HW exec time: {res.exec_time_ns} ns")
        _CACHE["exec_time_ns"] = res.exec_time_ns
        _CACHE["trace"] = res.instructions_and_trace

    out = np.zeros((B, T, D), dtype=np.float32)
    for c in range(NCORES):
        out[c // 4] += res.results[c]["y"].astype(np.float32)
    out += bo[None, None, :]
    return out


# revision 3
# speedup vs baseline: 1.7636x; 1.7636x over previous
"""Multi-head self-attention (B=2, T=4096, D=768, H=12) on 8 TRN2 NeuronCores.

Sharding: (batch, head)-parallel. Core c (0..7) handles batch b=c//4 and the
3 heads h0=(c%4)*3 .. h0+2.  Each core computes Q/K/V projections for its
heads, full softmax(QK^T/sqrt(d))V attention, and a partial output projection
through its 192 rows of Wo.  The host sums the 4 partials per batch and adds
the output bias bo.

Per-core pipeline (v2):
  Phase 1: Q/K projections into [d, t] layout (head pair packed on 128
  partitions; h2's 64-dim blocks of Q and K col-tiled into one PSUM bank);
  V in natural [t, d] layout augmented with a trailing ones column (row 64 of
  the PV output then accumulates the softmax denominator).
  Phase 2, per 512-wide q tile: for each 128-wide k tile, a row-tiled pair of
  score matmuls (S^T[k,q] for h0/h1 concurrently); exp alternates between the
  ACT engine (exact, PSUM->SBUF bf16) and the DVE (Schraudolph bitcast exp:
  i16 = round(23.083*s + 16250.5) whose bits are bf16 e^(s/8), ~3% pointwise,
  ~0.2% on the softmax output); PV accumulates O^T[65,512] per head with the
  denominator in row 64.  h2 is processed with its Q/K duplicated onto both
  row halves, two k tiles per step.  Normalization divides by row 64 via a
  K=1 broadcast matmul + fast reciprocal.  The Wo projection consumes the
  merged normalized tile on01 [128, t] (h0 rows 0:64, h1 rows 64:128) and
  on2 [64, t] in two accumulating matmuls per output block; y is written bf16.
"""

import os
import numpy as np
import ml_dtypes

B, T, D = 2, 4096, 768
H, DH = 12, 64
NCORES = 8
HPC = 3            # heads per core
KC = D // 128      # 6 contraction chunks for projections
NT = T // 512      # 8 q tiles of 512
TT = T // 128      # 32 k tiles of 128

# Schraudolph constants: i16 = round(s * SCH_A + SCH_B); bits viewed as bf16
# give e^(s/8) with ~3.3% max pointwise error.
SCH_A = 128 / float(np.log(2)) * 0.125   # 23.08312065
SCH_B = 16256.0 - 5.5

BF16 = ml_dtypes.bfloat16

_CACHE = {}


def _trace(nc, tc, mybir, tens, iters=1):
    import concourse.bass as bass
    from contextlib import ExitStack

    f32 = mybir.dt.float32
    bf16 = mybir.dt.bfloat16
    i16 = mybir.dt.int16
    f32r = mybir.dt.float32r
    Exp = mybir.ActivationFunctionType.Exp
    PSUM = bass.MemorySpace.PSUM
    Mult = mybir.AluOpType.mult
    Add = mybir.AluOpType.add

    with ExitStack() as ctx:
        persist = ctx.enter_context(tc.tile_pool(name="persist", bufs=1))

        # ---- persistent SBUF ----
        x_ch = [
            persist.tile([128, T], bf16, name=f"xc{kc}") for kc in range(KC)
        ]
        w_q = persist.tile([128, KC, HPC * DH], bf16)
        w_k = persist.tile([128, KC, HPC * DH], bf16)
        w_v = persist.tile([128, KC, HPC * DH], bf16)
        bq01 = persist.tile([128, 1], f32)
        bq2 = persist.tile([64, 1], f32)
        bk01 = persist.tile([128, 1], f32)
        bk2 = persist.tile([64, 1], f32)
        bv_sb = persist.tile([1, HPC * DH], bf16)
        ones1 = persist.tile([1, 128], bf16)     # K=1 lhsT for V bias MM
        ones65 = persist.tile([DH + 1, DH + 1], f32r)  # row 64: K=1 denom bcast lhsT
        q01 = persist.tile([128, T], bf16)       # h0 rows 0:64, h1 rows 64:128
        k01 = persist.tile([128, T], bf16)
        q2 = persist.tile([128, T], bf16)        # h2, duplicated to rows 64:128
        k2 = persist.tile([128, T], bf16)
        v_sb = persist.tile([128, TT, HPC, 68], bf16)  # [V|1] per head
        # normalized O^T: h0 rows 0:64, h1 rows 64:128; h2 separate
        on01 = persist.tile([128, T], bf16)
        on2 = persist.tile([DH, T], bf16)
        wo01_sb = persist.tile([128, D], bf16)
        wo2_sb = persist.tile([DH, D], bf16)

        nc.vector.memset(v_sb[:, :, :, 64:65], 1.0)

        # ---- input DMAs ----
        xT, wqT, wkT, wvT, bq, bk, bv, wo01, wo2, onesb, ones65d, y = tens
        nc.sync.dma_start(ones1[:], onesb[0:1, 0:128])
        nc.sync.dma_start(ones65[DH : DH + 1, :], ones65d[:])
        for kc in range(KC):
            r = slice(kc * 128, (kc + 1) * 128)
            nc.sync.dma_start(x_ch[kc][:], xT[r, :])
            nc.sync.dma_start(w_q[:, kc, :], wqT[r, :])
            nc.sync.dma_start(w_k[:, kc, :], wkT[r, :])
            nc.sync.dma_start(w_v[:, kc, :], wvT[r, :])
        nc.sync.dma_start(bq01[:], bq[0:128, :])
        nc.sync.dma_start(bq2[:], bq[128:192, :])
        nc.sync.dma_start(bk01[:], bk[0:128, :])
        nc.sync.dma_start(bk2[:], bk[128:192, :])
        nc.sync.dma_start(bv_sb[:], bv[:])
        nc.sync.dma_start(wo01_sb[:], wo01[:])
        nc.sync.dma_start(wo2_sb[:], wo2[:])

        loop_cm = tc.For_i(0, iters, 1) if iters > 1 else None
        from contextlib import nullcontext
        with (loop_cm if loop_cm is not None else nullcontext()):
            # ---- Phase 1a: Q/K projections into [d, t] layout ----
            with tc.tile_pool(name="pj", bufs=2, space=PSUM) as pj:
                for nt in range(NT):
                    s = slice(nt * 512, (nt + 1) * 512)
                    pqa = pj.tile([128, 512], f32, tag="pqa")
                    pka = pj.tile([128, 512], f32, tag="pka")
                    pb = pj.tile([128, 512], f32, tag="pb")  # q2 | k2 col-tiled
                    for kc in range(KC):
                        st, sp = kc == 0, kc == KC - 1
                        rhs = x_ch[kc][:, s]
                        nc.tensor.matmul(pqa[:], w_q[:, kc, 0:128], rhs, start=st, stop=sp)
                        nc.tensor.matmul(pka[:], w_k[:, kc, 0:128], rhs, start=st, stop=sp)
                        nc.tensor.matmul(pb[0:64, :], w_q[:, kc, 128:192], rhs,
                                         start=st, stop=sp, tile_position=(0, 0),
                                         skip_group_check=True)
                        nc.tensor.matmul(pb[64:128, :], w_k[:, kc, 128:192], rhs,
                                         start=st, stop=sp, tile_position=(0, 64),
                                         skip_group_check=True)
                    nc.vector.tensor_scalar_add(q01[:, s], pqa[:], bq01[:])
                    nc.vector.tensor_scalar_add(k01[:, s], pka[:], bk01[:])
                    nc.vector.tensor_scalar_add(q2[0:64, s], pb[0:64, :], bq2[:])
                    nc.vector.tensor_scalar_add(k2[0:64, s], pb[64:128, :], bk2[:])

            # ---- Phase 1b: V projection into natural [t, d] layout ----
            with tc.tile_pool(name="pv", bufs=4, space=PSUM) as pvp:
                for tt in range(TT):
                    ts_ = slice(tt * 128, (tt + 1) * 128)
                    pvt = pvp.tile([128, HPC * DH], f32, tag="pvt")
                    nc.tensor.matmul(pvt[:], ones1[:], bv_sb[:], start=True, stop=False)
                    for kc in range(KC):
                        nc.tensor.matmul(
                            pvt[:], x_ch[kc][:, ts_], w_v[:, kc, :],
                            start=False, stop=kc == KC - 1,
                        )
                    nc.vector.tensor_copy(
                        v_sb[:, tt, :, 0:64],
                        pvt[:].rearrange("p (h d) -> p h d", h=HPC),
                    )

            # duplicate h2's Q/K to partitions 64..127 for self-paired row tiling
            nc.sync.dma_start(q2[64:128, :], q2[0:64, :])
            nc.sync.dma_start(k2[64:128, :], k2[0:64, :])

            # ---- Phase 2: attention + output projection, per q tile ----
            with (
                tc.tile_pool(name="spool", bufs=2, space=PSUM) as spool,
                tc.tile_pool(name="opool", bufs=1, space=PSUM) as opool,
                tc.tile_pool(name="mpool", bufs=2, space=PSUM) as mpool,
                tc.tile_pool(name="ppool", bufs=3) as ppool,
                tc.tile_pool(name="npool", bufs=2) as npool,
                tc.tile_pool(name="ypool", bufs=2) as ypool,
            ):
                for qt in range(NT):
                    qs = slice(qt * 512, (qt + 1) * 512)

                    def exp_jobs(st_pair, act_idx):
                        """Two [128,512] exp jobs; act_idx goes to ACT, other to DVE."""
                        pt = [
                            ppool.tile([128, 512], bf16, tag=f"p{i}", name=f"p{i}")
                            for i in (0, 1)
                        ]
                        d_idx = 1 - act_idx
                        nc.scalar.activation(pt[act_idx][:], st_pair[act_idx][:],
                                             Exp, scale=0.125)
                        nc.vector.tensor_scalar(pt[d_idx][:].bitcast(i16),
                                                st_pair[d_idx][:],
                                                SCH_A, SCH_B, Mult, Add)
                        return pt

                    # --- h0/h1 concurrently (row strips 0:64 / 64:128) ---
                    ol = [opool.tile([DH + 1, 512], f32, tag=f"o{i}", name=f"ol{i}")
                          for i in (0, 1)]
                    for kt in range(TT):
                        ks = slice(kt * 128, (kt + 1) * 128)
                        st = [spool.tile([128, 512], f32, tag=f"s{i}", name=f"s{i}")
                              for i in (0, 1)]
                        nc.tensor.matmul(st[0][:], k01[0:64, ks], q01[0:64, qs],
                                         start=True, stop=True)
                        nc.tensor.matmul(st[1][:], k01[64:128, ks], q01[64:128, qs],
                                         start=True, stop=True)
                        pt = exp_jobs(st, kt % 2)
                        first, last = kt == 0, kt == TT - 1
                        nc.tensor.matmul(ol[0][:], v_sb[:, kt, 0, 0:65], pt[0][:],
                                         start=first, stop=last, skip_group_check=True)
                        nc.tensor.matmul(ol[1][:], v_sb[:, kt, 1, 0:65], pt[1][:],
                                         start=first, stop=last, skip_group_check=True)

                    # --- normalize h0/h1 into merged on01 ---
                    def norm(o_acc, out_ap):
                        lrow = npool.tile([DH + 1, 512], f32r, tag="lr")
                        nc.vector.tensor_copy(lrow[DH : DH + 1, :], o_acc[DH : DH + 1, :])
                        bc = mpool.tile([DH + 1, 512], f32, tag="y")
                        nc.tensor.matmul(bc[:], ones65[DH : DH + 1, :],
                                         lrow[DH : DH + 1, :], start=True, stop=True)
                        rc = npool.tile([DH + 1, 512], f32, tag="rc")
                        nc.vector.reciprocal_approx_fast(rc[:], bc[:])
                        nc.vector.tensor_mul(out_ap, o_acc[0:DH, :], rc[0:DH, :])

                    norm(ol[0], on01[0:DH, qs])
                    norm(ol[1], on01[DH:128, qs])

                    # --- h2: two k tiles per step on row halves ---
                    o2 = [opool.tile([DH + 1, 512], f32, tag=f"o{i}", name=f"o2_{i}")
                          for i in (0, 1)]
                    for p in range(TT // 2):
                        ka = slice(2 * p * 128, (2 * p + 1) * 128)
                        kb = slice((2 * p + 1) * 128, (2 * p + 2) * 128)
                        st = [spool.tile([128, 512], f32, tag=f"s{i}", name=f"s2{i}")
                              for i in (0, 1)]
                        nc.tensor.matmul(st[0][:], k2[0:64, ka], q2[0:64, qs],
                                         start=True, stop=True)
                        nc.tensor.matmul(st[1][:], k2[64:128, kb], q2[64:128, qs],
                                         start=True, stop=True)
                        pt = exp_jobs(st, p % 2)
                        first, last = p == 0, p == TT // 2 - 1
                        nc.tensor.matmul(o2[0][:], v_sb[:, 2 * p, 2, 0:65], pt[0][:],
                                         start=first, stop=last, skip_group_check=True)
                        nc.tensor.matmul(o2[1][:], v_sb[:, 2 * p + 1, 2, 0:65], pt[1][:],
                                         start=first, stop=last, skip_group_check=True)

                    o2b = npool.tile([DH + 1, 512], f32, tag="o2b")
                    nc.vector.tensor_copy(o2b[:], o2[1][:])
                    o2s = npool.tile([DH + 1, 512], f32, tag="o2s")
                    nc.vector.tensor_add(o2s[:], o2[0][:], o2b[:])
                    norm(o2s, on2[0:DH, qs])

                    # --- output projection for this q tile ---
                    for tt4 in range(4):
                        t0 = qt * 512 + tt4 * 128
                        ts_ = slice(t0, t0 + 128)
                        ysb = ypool.tile([128, D], bf16, tag="ysb")
                        for m0, mw in ((0, 512), (512, 256)):
                            ms = slice(m0, m0 + mw)
                            yps = mpool.tile([128, 512], f32, tag="y")
                            nc.tensor.matmul(yps[:, 0:mw], on01[:, ts_], wo01_sb[:, ms],
                                             start=True, stop=False)
                            nc.tensor.matmul(yps[:, 0:mw], on2[:, ts_], wo2_sb[:, ms],
                                             start=False, stop=True)
                            nc.vector.tensor_copy(ysb[:, ms], yps[:, 0:mw])
                        nc.sync.dma_start(y[ts_, :], ysb[:])


def _build(iters=1):
    import concourse.bacc as bacc
    import concourse.tile as tile
    from concourse import mybir

    f32 = mybir.dt.float32
    bf16 = mybir.dt.bfloat16
    nc = bacc.Bacc("TRN2", target_bir_lowering=False, debug=False, name="mhsa")

    tens = (
        nc.dram_tensor("xT", [D, T], bf16, kind="ExternalInput"),
        nc.dram_tensor("wqT", [D, HPC * DH], bf16, kind="ExternalInput"),
        nc.dram_tensor("wkT", [D, HPC * DH], bf16, kind="ExternalInput"),
        nc.dram_tensor("wvT", [D, HPC * DH], bf16, kind="ExternalInput"),
        nc.dram_tensor("bq", [HPC * DH, 1], f32, kind="ExternalInput"),
        nc.dram_tensor("bk", [HPC * DH, 1], f32, kind="ExternalInput"),
        nc.dram_tensor("bv", [1, HPC * DH], bf16, kind="ExternalInput"),
        nc.dram_tensor("wo01", [128, D], bf16, kind="ExternalInput"),
        nc.dram_tensor("wo2", [DH, D], bf16, kind="ExternalInput"),
        nc.dram_tensor("onesb", [1, T], bf16, kind="ExternalInput"),
        nc.dram_tensor("ones65", [1, DH + 1], mybir.dt.float32r, kind="ExternalInput"),
        nc.dram_tensor("y", [T, D], bf16, kind="ExternalOutput"),
    )
    with tile.TileContext(nc) as tc:
        _trace(nc, tc, mybir, tens, iters)
    nc.finalize()
    return nc


def _prep_inputs(x, Wq, bq, Wk, bk, Wv, bv, Wo, bo):
    in_maps = []
    xTb = [np.ascontiguousarray(x[b].T).astype(BF16) for b in range(B)]
    for c in range(NCORES):
        b = c // 4
        h0 = (c % 4) * HPC
        cols = slice(h0 * DH, (h0 + HPC) * DH)
        woT = np.ascontiguousarray(Wo[:, cols].T)  # [192, 768]
        wo01 = np.ascontiguousarray(woT[0:128]).astype(BF16)
        wo2 = np.ascontiguousarray(woT[128:192]).astype(BF16)
        in_maps.append(
            {
                "xT": xTb[b],
                "wqT": np.ascontiguousarray(Wq[cols, :].T).astype(BF16),
                "wkT": np.ascontiguousarray(Wk[cols, :].T).astype(BF16),
                "wvT": np.ascontiguousarray(Wv[cols, :].T).astype(BF16),
                "bq": np.ascontiguousarray(bq[cols]).reshape(-1, 1).astype(np.float32),
                "bk": np.ascontiguousarray(bk[cols]).reshape(-1, 1).astype(np.float32),
                "bv": np.ascontiguousarray(bv[cols]).reshape(1, -1).astype(BF16),
                "wo01": wo01,
                "wo2": wo2,
                "onesb": np.ones((1, T), dtype=BF16),
                "ones65": np.ones((1, DH + 1), dtype=np.float32),
            }
        )
    return in_maps


def kernel(x, Wq, bq, Wk, bk, Wv, bv, Wo, bo):
    x = np.asarray(x, dtype=np.float32)
    Wq, bq = np.asarray(Wq, np.float32), np.asarray(bq, np.float32)
    Wk, bk = np.asarray(Wk, np.float32), np.asarray(bk, np.float32)
    Wv, bv = np.asarray(Wv, np.float32), np.asarray(bv, np.float32)
    Wo, bo = np.asarray(Wo, np.float32), np.asarray(bo, np.float32)

    from concourse.bass_utils import run_bass_kernel_spmd

    iters = int(os.environ.get("MHSA_ITERS", "1"))
    key = ("nc", iters)
    if key not in _CACHE:
        _CACHE[key] = _build(iters)
    nc = _CACHE[key]

    in_maps = _prep_inputs(x, Wq, bq, Wk, bk, Wv, bv, Wo, bo)
    trace = bool(os.environ.get("MHSA_TRACE"))
    ncores = int(os.environ.get("MHSA_NCORES", NCORES))
    res = run_bass_kernel_spmd(
        nc, in_maps[:ncores], core_ids=list(range(ncores)), trace=trace
    )
    if res.exec_time_ns is not None:
        print(f"HW exec time: {res.exec_time_ns} ns")
        _CACHE["exec_time_ns"] = res.exec_time_ns
        _CACHE["trace"] = res.instructions_and_trace

    out = np.zeros((B, T, D), dtype=np.float32)
    for c in range(ncores):
        out[c // 4] += res.results[c]["y"].astype(np.float32)
    out += bo[None, None, :]
    return out
